# revision 24
# baseline (speedup 1.0000x reference)
"""Trainium2 Bass kernel for a 4-layer dense transformer (B=2, S=1024, D=1024, H=16).

Sharding: context-parallel over tokens across 8 cores (256 tokens/core;
cores 0-3 = batch 0, cores 4-7 = batch 1). Per layer, K/V are exchanged
within each 4-core batch group via one AllGather; everything else is local.

On-chip layout: feature-major residual h^T [D, T] so every GEMM consumes
weights in native [in, out] layout as the stationary operand with zero
transposes. Scores are computed as S^T [k, q]; softmax runs over the
partition axis. The softmax denominator comes for free from a scaled
ones-column appended to each head's V block (psum row 64). K/V cross the
wire in fp8 e3m4 scaled by 4 (folded into wk/wv); GEMM operands are
otherwise fp16 (fp32 PSUM accumulate); residual and LN stats stay fp32.
"""

import sys
import os

for _p in ("/opt/trn_rl_repo", "/root/.axon_site/_ro/trn_rl_repo"):
    if os.path.isdir(_p) and _p not in sys.path:
        sys.path.insert(0, _p)

import numpy as np
import concourse.bass as bass
import concourse.bacc as bacc
import concourse.mybir as mybir
import concourse.tile as tile
from concourse.bass_utils import run_bass_kernel_spmd

dt = mybir.dt
AF = mybir.ActivationFunctionType
ALU = mybir.AluOpType

L, B, S, D, H = 4, 2, 1024, 1024, 16
DH = D // H
F = 4 * D
ROPE_BASE = 10000.0
LN_EPS = 1e-5

N_CORES = 8
T = (B * S) // N_CORES            # 256 tokens per core
DC = D // 128                     # 8 feature chunks
HP = H // 2                       # 8 head pairs
GROUPS = [[0, 1, 2, 3], [4, 5, 6, 7]]
RANKS = 4                         # cores per batch group
KV_K = D * T                      # elems of local K^T block
KV_TOT = 2 * KV_K                 # K^T + V per core
KV_SCALE = 4.0                    # fp8 wire scale for K and V (folded in)
VW = DH + 1                       # V block width per head incl. ones col

_SHUF_MASK = [(i + 16) % 32 for i in range(32)]


def _qk_perm():
    """Per-head permutation: [16 even-rows; 16 odd-rows] per 32-row quadrant."""
    perm = np.zeros(D, dtype=np.int64)
    for h in range(H):
        for quad in range(2):
            for j in range(32):
                pair = quad * 16 + (j % 16)
                old_d = 2 * pair + (1 if j >= 16 else 0)
                perm[h * 64 + quad * 32 + j] = h * 64 + old_d
    return perm


def _rope_tables(core):
    """cos [128,2T] fp32 and signed-sin [128,2T] fp16 (tables doubled so a
    pair of feature chunks shares one vector op)."""
    j = core % RANKS
    pos = j * T + np.arange(T, dtype=np.float64)
    inv_freq = 1.0 / (ROPE_BASE ** (np.arange(0, DH, 2, dtype=np.float64) / DH))
    cos128 = np.zeros((128, T), dtype=np.float32)
    ss128 = np.zeros((128, T), dtype=np.float32)
    for p in range(128):
        qq, jj = p // 32, p % 32
        i = (qq % 2) * 16 + (jj % 16)
        ang = pos * inv_freq[i]
        cos128[p] = np.cos(ang)
        ss128[p] = (-np.sin(ang)) if jj < 16 else np.sin(ang)
    cos2 = np.concatenate([cos128, cos128], axis=1)
    ss2 = np.concatenate([ss128, ss128], axis=1).astype(np.float16)
    return cos2, ss2


def _causal_mask(core):
    """maskT [128, DC*T] fp16: mask[p, kc*T + t] = key kc*128+p visible to query t."""
    j = core % RANKS
    q = j * T + np.arange(T)
    m = np.zeros((128, DC * T), dtype=np.float16)
    for kc in range(DC):
        k = kc * 128 + np.arange(128)
        m[:, kc * T:(kc + 1) * T] = (k[:, None] <= q[None, :]).astype(np.float16)
    return m


def build_program():
    nc = bacc.Bacc("TRN2", target_bir_lowering=False, debug=False,
                   num_devices=N_CORES)
    f16, f32, f8 = dt.float16, dt.float32, dt.float8e3

    x0T = nc.dram_tensor("x0T", [D, T], f32, kind="ExternalInput")
    cosT = nc.dram_tensor("cosT", [128, 2 * T], f32, kind="ExternalInput")
    ssT = nc.dram_tensor("ssT", [128, 2 * T], f16, kind="ExternalInput")
    maskT = nc.dram_tensor("maskT", [128, DC * T], f16, kind="ExternalInput")
    wqk = nc.dram_tensor("wqk", [L, D, 2 * D], f16, kind="ExternalInput")
    wv = nc.dram_tensor("wv", [L, D, D], f16, kind="ExternalInput")
    wproj = nc.dram_tensor("wproj", [L, D, D], f16, kind="ExternalInput")
    wfc = nc.dram_tensor("wfc", [L, D, F], f16, kind="ExternalInput")
    wout = nc.dram_tensor("wout", [L, F, D], f16, kind="ExternalInput")
    biases = nc.dram_tensor("biases", [L, 128, 64], f32, kind="ExternalInput")
    bv = nc.dram_tensor("bv", [L, 1, D], f16, kind="ExternalInput")
    lnfg = nc.dram_tensor("lnfg", [128, 8], f32, kind="ExternalInput")
    lnfb = nc.dram_tensor("lnfb", [128, 8], f32, kind="ExternalInput")
    outT = nc.dram_tensor("outT", [D, T], f32, kind="ExternalOutput")

    from contextlib import ExitStack
    with ExitStack() as _es:
        tc = _es.enter_context(tile.TileContext(nc))
        pp = _es.enter_context(tc.tile_pool(name="persist", bufs=1))
        wqk_pool = _es.enter_context(tc.tile_pool(name="wqk", bufs=1))
        wv_pool = _es.enter_context(tc.tile_pool(name="wv", bufs=1))
        wpr_pool = _es.enter_context(tc.tile_pool(name="wpr", bufs=1))
        wfc_pool = _es.enter_context(tc.tile_pool(name="wfc", bufs=3))
        wout_pool = _es.enter_context(tc.tile_pool(name="wout", bufs=2))
        bias_pool = _es.enter_context(tc.tile_pool(name="bias", bufs=2))
        xh_pool = _es.enter_context(tc.tile_pool(name="xh", bufs=1))
        t16_pool = _es.enter_context(tc.tile_pool(name="ln16", bufs=2))
        rope_pool = _es.enter_context(tc.tile_pool(name="rope", bufs=2))
        t32_pool = _es.enter_context(tc.tile_pool(name="tmp32", bufs=2))
        probs_pool = _es.enter_context(tc.tile_pool(name="probs", bufs=3))
        stat_pool = _es.enter_context(tc.tile_pool(name="stat", bufs=1))
        bc_pool = _es.enter_context(tc.tile_pool(name="bcast", bufs=2))
        ps_small = _es.enter_context(tc.tile_pool(name="ps_small", bufs=1, space="PSUM"))
        ps_bank = _es.enter_context(tc.tile_pool(name="ps_bank", bufs=5, space="PSUM"))
        ps_at = _es.enter_context(tc.tile_pool(name="ps_at", bufs=2, space="PSUM"))
        dram = _es.enter_context(tc.tile_pool(name="dram", bufs=1, space="DRAM"))
        if True:
            h_sb = pp.tile([128, DC * T], f32)
            cos_sb = pp.tile([128, 2 * T], f32)
            ss_sb = pp.tile([128, 2 * T], f16)
            mask_sb = pp.tile([128, DC * T], f16)
            Q_sb = pp.tile([128, HP * T], f16)
            Kl_sb = pp.tile([128, HP * T], f8)
            Vl_sb = pp.tile([128, 2 * D], f8)
            K_sb = pp.tile([128, HP * S], f8)
            V_sb = pp.tile([128, DC * H * VW], f8)
            attn_sb = pp.tile([128, DC * T], f16)
            h1_sb = pp.tile([128, (F // 128) * T], f16)  # [128, 8192]
            ones_c = pp.tile([128, 1], f16)
            ones_r = pp.tile([1, 128], f16)
            eps_c = pp.tile([1, 1], f32)
            lnfg_sb = pp.tile([128, 8], f32)
            lnfb_sb = pp.tile([128, 8], f32)

            kvloc = dram.tile([KV_TOT], f8)
            kvag = dram.tile([RANKS * KV_TOT], f8)

            nc.vector.memset(ones_c[:], 1.0)
            nc.vector.memset(ones_r[:], 1.0)
            nc.vector.memset(eps_c[:], LN_EPS)
            # ones columns (col DH of each head block) give the softmax
            # denominator; the V unpack DMAs only ever write cols 0..DH-1.
            nc.vector.memset(V_sb[:], KV_SCALE)
            nc.sync.dma_start(out=cos_sb[:], in_=cosT[:])
            nc.sync.dma_start(out=ss_sb[:], in_=ssT[:])
            nc.sync.dma_start(out=mask_sb[:], in_=maskT[:])
            nc.sync.dma_start(out=lnfg_sb[:], in_=lnfg[:])
            nc.sync.dma_start(out=lnfb_sb[:], in_=lnfb[:])
            for ci in range(DC):
                nc.sync.dma_start(
                    out=h_sb[:, ci * T:(ci + 1) * T],
                    in_=x0T[ci * 128:(ci + 1) * 128, :],
                )

            def load_qkv_weights(l):
                wqk_t = wqk_pool.tile([128, DC * 2 * D], f16, tag="wqk")
                wv_t = wv_pool.tile([128, DC * D], f16, tag="wv")
                nc.sync.dma_start(
                    out=wqk_t[:].rearrange("p (c n) -> p c n", c=DC),
                    in_=wqk[l].rearrange("(c p) n -> p c n", p=128),
                )
                nc.sync.dma_start(
                    out=wv_t[:].rearrange("p (c n) -> p c n", c=DC),
                    in_=wv[l].rearrange("(c p) n -> p c n", p=128),
                )
                return wqk_t, wv_t

            def ln_stats_chunk(p_ss, ci):
                """Accumulate sum(h) | sum(h^2) for chunk ci into p_ss."""
                hc = h_sb[:, ci * T:(ci + 1) * T]
                hsq = t16_pool.tile([128, 2 * T], f16, tag="hsq")
                nc.vector.tensor_copy(hsq[:, 0:T], hc)
                nc.scalar.activation(hsq[:, T:2 * T], hc, AF.Square)
                nc.tensor.matmul(p_ss[:], ones_c[:], hsq[:],
                                 start=(ci == 0), stop=(ci == DC - 1))

            def layer_norm(xhat, gb=None, p_ss=None):
                """h_sb (f32) -> xhat (f16 [128, DC*T]) normalized. If gb is
                given, apply per-feature gamma/beta (final LN, f32 out). If
                p_ss is given, the stats were already accumulated inline."""
                if p_ss is None:
                    p_ss = ps_small.tile([1, 2 * T], f32, tag="ps_small")
                    for ci in range(DC):
                        ln_stats_chunk(p_ss, ci)
                m = stat_pool.tile([1, T], f32, tag="st_m")
                msq = stat_pool.tile([1, T], f32, tag="st_msq")
                var = stat_pool.tile([1, T], f32, tag="st_var")
                rstd = stat_pool.tile([1, T], f32, tag="st_rstd")
                mr = stat_pool.tile([1, T], f32, tag="st_mr")
                nc.vector.tensor_scalar_mul(m[:], p_ss[:, 0:T], 1.0 / D)
                nc.vector.tensor_scalar_mul(msq[:], p_ss[:, T:2 * T], 1.0 / D)
                nc.vector.tensor_tensor(out=var[:], in0=m[:], in1=m[:], op=ALU.mult)
                nc.vector.tensor_sub(var[:], msq[:], var[:])
                # rstd = exp(-0.5 * ln(var + eps)) — keeps ACT on the ln/exp table
                nc.scalar.activation(var[:], var[:], AF.Ln, bias=eps_c[:])
                nc.vector.tensor_scalar_mul(var[:], var[:], -0.5)
                nc.scalar.activation(rstd[:], var[:], AF.Exp)
                nc.vector.tensor_tensor(out=mr[:], in0=m[:], in1=rstd[:], op=ALU.mult)
                rstd_b = bc_pool.tile([128, T], f32, tag="rstd_b")
                mr_b = bc_pool.tile([128, T], f32, tag="mr_b")
                nc.gpsimd.partition_broadcast(rstd_b[:], rstd[:])
                nc.gpsimd.partition_broadcast(mr_b[:], mr[:])
                for ci in range(DC):
                    hc = h_sb[:, ci * T:(ci + 1) * T]
                    u = t32_pool.tile([128, T], f32, tag="ln_u")
                    nc.vector.tensor_tensor(out=u[:], in0=hc, in1=rstd_b[:],
                                            op=ALU.mult)
                    if gb is None:
                        nc.vector.tensor_tensor(out=xhat[:, ci * T:(ci + 1) * T],
                                                in0=u[:], in1=mr_b[:],
                                                op=ALU.subtract)
                    else:
                        g_sb, b_sb = gb
                        z = t32_pool.tile([128, T], f32, tag="ln_z")
                        nc.vector.tensor_tensor(out=z[:], in0=u[:], in1=mr_b[:],
                                                op=ALU.subtract)
                        nc.vector.tensor_scalar(
                            out=xhat[:, ci * T:(ci + 1) * T], in0=z[:],
                            scalar1=g_sb[:, ci:ci + 1], scalar2=b_sb[:, ci:ci + 1],
                            op0=ALU.mult, op1=ALU.add,
                        )

            def rope_pair(p_qk, dest, pi, bqk_t, bias_off):
                """p_qk [128,2T] psum holding chunk pair (2pi, 2pi+1); write
                RoPE'd pair into dest[:, 2pi*T:(2pi+2)*T]."""
                qtmp = rope_pool.tile([128, 2 * T], f16, tag="rope_q")
                ctmp = rope_pool.tile([128, 2 * T], f16, tag="rope_c")
                stmp = rope_pool.tile([128, 2 * T], f16, tag="rope_s")
                dtmp = rope_pool.tile([128, 2 * T], f16, tag="rope_d")
                for half in range(2):
                    bcol = bqk_t[:, bias_off + 2 * pi + half:bias_off + 2 * pi + half + 1]
                    sl = slice(half * T, (half + 1) * T)
                    nc.vector.tensor_scalar_add(qtmp[:, sl], p_qk[:, sl], bcol)
                    nc.vector.scalar_tensor_tensor(
                        out=ctmp[:, sl], in0=p_qk[:, sl], scalar=bcol,
                        in1=cos_sb[:, sl], op0=ALU.add, op1=ALU.mult,
                    )
                nc.vector.stream_shuffle(stmp[:], qtmp[:], _SHUF_MASK)
                nc.vector.tensor_tensor(out=dtmp[:], in0=stmp[:], in1=ss_sb[:],
                                        op=ALU.mult)
                nc.vector.tensor_tensor(
                    out=dest[:, 2 * pi * T:(2 * pi + 2) * T],
                    in0=ctmp[:], in1=dtmp[:], op=ALU.add,
                )

            def load_biases(l):
                bias_t = bias_pool.tile([128, 64], f32, tag="biases")
                bv_t = bias_pool.tile([1, D], f16, tag="bv")
                nc.sync.dma_start(out=bias_t[:], in_=biases[l])
                nc.sync.dma_start(out=bv_t[:], in_=bv[l])
                return bias_t, bv_t

            wqk_t, wv_t = load_qkv_weights(0)
            bias_tiles = {0: load_biases(0)}

            for l in range(L):
                # ---- per-layer bias tiles (prefetched) + proj weights ----
                bias_t, bv_t = bias_tiles[l]
                bqk_t = bias_t[:, 0:16]
                bproj_t = bias_t[:, 16:24]
                bfc_t = bias_t[:, 24:56]
                bout_t = bias_t[:, 56:64]
                wproj_t = wpr_pool.tile([128, DC * D], f16, tag="wproj")
                nc.sync.dma_start(
                    out=wproj_t[:].rearrange("p (c n) -> p c n", c=DC),
                    in_=wproj[l].rearrange("(c p) n -> p c n", p=128),
                )

                # ---- LN1 ----
                with nc.named_scope("ln1"):
                    xhat = xh_pool.tile([128, DC * T], f16, tag="xhat")
                    layer_norm(xhat, p_ss=(None if l == 0 else p_ss1))

                # ---- K projection + RoPE (first, so the gather launches early)
                with nc.named_scope("kv_proj"):
                    for pi in range(4):      # k chunk pairs
                        p_qk = ps_bank.tile([128, 2 * T], f32, tag="ps_bank")
                        for half in range(2):
                            fci = HP + 2 * pi + half
                            for dci in range(DC):
                                nc.tensor.matmul(
                                    p_qk[:, half * T:(half + 1) * T],
                                    wqk_t[:, dci * 2 * D + fci * 128:
                                      dci * 2 * D + (fci + 1) * 128],
                                    xhat[:, dci * T:(dci + 1) * T],
                                    start=(dci == 0), stop=(dci == DC - 1),
                                )
                        rope_pair(p_qk, Kl_sb, pi, bqk_t, HP)
                    # v: token-major [T, D] via lhsT = xhat slices
                    for tci in range(2):
                        for fh in range(2):
                            p_v = ps_bank.tile([128, 512], f32, tag="ps_bank")
                            for dci in range(DC):
                                nc.tensor.matmul(
                                    p_v[:],
                                    xhat[:, dci * T + tci * 128: dci * T + (tci + 1) * 128],
                                    wv_t[:, dci * D + fh * 512:
                                         dci * D + (fh + 1) * 512],
                                    start=(dci == 0), stop=False,
                                )
                            nc.tensor.matmul(
                                p_v[:], ones_r[:], bv_t[:, fh * 512:(fh + 1) * 512],
                                start=False, stop=True,
                            )
                            nc.vector.tensor_copy(
                                Vl_sb[:, tci * D + fh * 512: tci * D + (fh + 1) * 512],
                                p_v[:],
                            )

                # ---- stage K^T,V to DRAM; AllGather within batch group ----
                with nc.named_scope("kv_gather"):
                    for pi in range(4):
                        nc.sync.dma_start(
                            out=kvloc[2 * pi * T * 128:(2 * pi + 2) * T * 128]
                                .rearrange("(c p t) -> p c t", p=128, t=T),
                            in_=Kl_sb[:, 2 * pi * T:(2 * pi + 2) * T]
                                .rearrange("p (c t) -> p c t", t=T),
                        )
                    nc.sync.dma_start(
                        out=kvloc[KV_K:KV_TOT].rearrange(
                            "(c p f) -> p c f", p=128, f=D),
                        in_=Vl_sb[:].rearrange("p (c f) -> p c f", f=D),
                    )
                    nc.gpsimd.collective_compute(
                        "AllGather",
                        ALU.bypass,
                        ins=[kvloc.opt()],
                        outs=[kvag.opt()],
                        replica_groups=GROUPS,
                    )

                def issue_wfc(g):
                    t = wfc_pool.tile([128, DC * 512], f16, tag="wfc")
                    nc.sync.dma_start(
                        out=t[:].rearrange("p (c n) -> p c n", c=DC),
                        in_=wfc[l].rearrange("(c p) n -> p c n", p=128)
                            [:, :, g * 512:(g + 1) * 512],
                    )
                    return t

                def issue_wout(i):
                    half, fcg = i // 4, i % 4
                    t = wout_pool.tile([128, 8 * 512], f16, tag="wout")
                    nc.sync.dma_start(
                        out=t[:].rearrange("p (c n) -> p c n", c=8),
                        in_=wout[l].rearrange("(c p) n -> p c n", p=128)
                            [:, fcg * 8:(fcg + 1) * 8,
                             half * 512:(half + 1) * 512],
                    )
                    return t

                wfc_ts = {g: issue_wfc(g) for g in range(3)}
                wout_ts = {i: issue_wout(i) for i in range(2)}

                # ---- Q projection + RoPE (overlaps the gather) ----
                with nc.named_scope("q_proj"):
                    for pi in range(4):      # q chunk pairs
                        p_qk = ps_bank.tile([128, 2 * T], f32, tag="ps_bank")
                        for half in range(2):
                            fci = 2 * pi + half
                            for dci in range(DC):
                                nc.tensor.matmul(
                                    p_qk[:, half * T:(half + 1) * T],
                                    wqk_t[:, dci * 2 * D + fci * 128:
                                      dci * 2 * D + (fci + 1) * 128],
                                    xhat[:, dci * T:(dci + 1) * T],
                                    start=(dci == 0), stop=(dci == DC - 1),
                                )
                        rope_pair(p_qk, Q_sb, pi, bqk_t, 0)

                # ---- unpack gathered K/V ----
                with nc.named_scope("kv_unpack"):
                    for rr in range(RANKS):
                        base = rr * KV_TOT
                        nc.sync.dma_start(
                            out=K_sb[:, rr * HP * T:(rr + 1) * HP * T].rearrange(
                                "p (c t) -> p c t", t=T),
                            in_=kvag[base:base + KV_K].rearrange(
                                "(c p t) -> p c t", p=128, t=T),
                        )
                        for tci in range(2):
                            vbase = base + KV_K + tci * D * 128
                            nc.sync.dma_start(
                                out=V_sb[:].rearrange(
                                    "p (c h x) -> p c h x", h=H, x=VW
                                )[:, 2 * rr + tci, :, 0:DH],
                                in_=kvag[vbase:vbase + D * 128].rearrange(
                                    "(p h d) -> p h d", p=128, d=DH),
                            )

                # ---- attention ----
                with nc.named_scope("attn"):
                    for hp in range(HP):
                        p_at = ps_at.tile([VW, 2 * T], f32, tag="ps_at")
                        for hh in range(2):
                            bp = hh * 64
                            hglob = 2 * hp + hh
                            for kp in range(DC // 2):
                                p_s = ps_bank.tile([128, 2 * T], f32, tag="ps_bank")
                                for half in range(2):
                                    kc = 2 * kp + half
                                    koff = ((kc // 2) * HP * T + hp * T
                                            + (kc % 2) * 128)
                                    nc.tensor.matmul(
                                        p_s[:, half * T:(half + 1) * T],
                                        K_sb[bp:bp + 64, koff:koff + 128],
                                        Q_sb[bp:bp + 64, hp * T:(hp + 1) * T],
                                        start=True, stop=True,
                                    )
                                probs = probs_pool.tile([128, 2 * T], f16, tag="probs")
                                nc.scalar.activation(probs[:], p_s[:], AF.Exp)
                                nc.vector.tensor_tensor(
                                    out=probs[:], in0=probs[:],
                                    in1=mask_sb[:, 2 * kp * T:(2 * kp + 2) * T],
                                    op=ALU.mult,
                                )
                                for half in range(2):
                                    kc = 2 * kp + half
                                    nc.tensor.matmul(
                                        p_at[0:VW, hh * T:(hh + 1) * T],
                                        V_sb[:, kc * H * VW + hglob * VW:
                                             kc * H * VW + (hglob + 1) * VW],
                                        probs[:, half * T:(half + 1) * T],
                                        start=(kc == 0), stop=(kc == DC - 1),
                                    )
                        for hh in range(2):
                            recip = stat_pool.tile([1, T], f32, tag="recip")
                            nc.vector.reciprocal(
                                recip[:], p_at[DH:DH + 1, hh * T:(hh + 1) * T])
                            rb = bc_pool.tile([64, T], f32, tag="rb")
                            nc.gpsimd.partition_broadcast(rb[:], recip[:])
                            nc.vector.tensor_tensor(
                                out=attn_sb[hh * 64:(hh + 1) * 64,
                                            hp * T:(hp + 1) * T],
                                in0=p_at[0:64, hh * T:(hh + 1) * T],
                                in1=rb[:],
                                op=ALU.mult,
                            )

                # ---- attention out-proj + residual ----
                with nc.named_scope("proj"):
                    p_ss2 = ps_small.tile([1, 2 * T], f32, tag="ps_small")
                    for half in range(2):
                        p_pr = [ps_bank.tile([128, 2 * T], f32, tag="ps_bank",
                                             name=f"p_pr{dj}") for dj in range(2)]
                        for dj in range(4):
                            for cin in range(DC):
                                off = cin * D + half * 512 + dj * 128
                                nc.tensor.matmul(
                                    p_pr[dj // 2][:, (dj % 2) * T:(dj % 2 + 1) * T],
                                    wproj_t[:, off:off + 128],
                                    attn_sb[:, cin * T:(cin + 1) * T],
                                    start=(cin == 0), stop=(cin == DC - 1),
                                )
                        for dj in range(4):
                            dci = half * 4 + dj
                            nc.vector.scalar_tensor_tensor(
                                out=h_sb[:, dci * T:(dci + 1) * T],
                                in0=p_pr[dj // 2][:, (dj % 2) * T:(dj % 2 + 1) * T],
                                scalar=bproj_t[:, dci:dci + 1],
                                in1=h_sb[:, dci * T:(dci + 1) * T],
                                op0=ALU.add, op1=ALU.add,
                            )
                            ln_stats_chunk(p_ss2, dci)

                # ---- LN2 ----
                with nc.named_scope("ln2"):
                    xhat2 = xh_pool.tile([128, DC * T], f16, tag="xhat")
                    layer_norm(xhat2, p_ss=p_ss2)

                # prefetch next layer's qkv weights + biases (overlaps the FFN)
                if l + 1 < L:
                    wqk_t, wv_t = load_qkv_weights(l + 1)
                    bias_tiles[l + 1] = load_biases(l + 1)

                # ---- FFN: fc + gelu -> h1, then out-proj + residual ----
                with nc.named_scope("fc"):
                    for g in range(F // 512):          # 8 groups of 4 output chunks
                        p_fc = [ps_bank.tile([128, 2 * T], f32, tag="ps_bank",
                                             name=f"p_fc{fj}") for fj in range(2)]
                        wfc_t = wfc_ts[g]
                        if g + 3 < F // 512:
                            wfc_ts[g + 3] = issue_wfc(g + 3)
                        for fj in range(4):
                            for dci in range(DC):
                                nc.tensor.matmul(
                                    p_fc[fj // 2][:, (fj % 2) * T:(fj % 2 + 1) * T],
                                    wfc_t[:, dci * 512 + fj * 128:
                                          dci * 512 + (fj + 1) * 128],
                                    xhat2[:, dci * T:(dci + 1) * T],
                                    start=(dci == 0), stop=(dci == DC - 1),
                                )
                        for fj in range(4):
                            fci = g * 4 + fj
                            nc.scalar.activation(
                                h1_sb[:, fci * T:(fci + 1) * T],
                                p_fc[fj // 2][:, (fj % 2) * T:(fj % 2 + 1) * T],
                                AF.Gelu_apprx_tanh,
                                bias=bfc_t[:, fci:fci + 1],
                            )
                with nc.named_scope("ffn_out"):
                    warm = stat_pool.tile([1, 1], f32, tag="warm")
                    nc.scalar.activation(warm[:], eps_c[:], AF.Ln)
                    p_ss1 = ps_small.tile([1, 2 * T], f32, tag="ps_small")
                    for half in range(2):
                        p_o = [ps_bank.tile([128, 2 * T], f32, tag="ps_bank",
                                            name=f"p_o{dj}") for dj in range(4)]
                        for fcg in range(4):       # 32 contraction chunks in 4 groups
                            i = half * 4 + fcg
                            wout_t = wout_ts[i]
                            if i + 2 < 8:
                                wout_ts[i + 2] = issue_wout(i + 2)
                            for fcl in range(8):
                                fci = fcg * 8 + fcl
                                for dj in range(4):
                                    nc.tensor.matmul(
                                        p_o[dj][:, 0:T],
                                        wout_t[:, fcl * 512 + dj * 128:
                                               fcl * 512 + (dj + 1) * 128],
                                        h1_sb[:, fci * T:(fci + 1) * T],
                                        start=(fci == 0), stop=(fci == F // 128 - 1),
                                    )
                        for dj in range(4):
                            dci = half * 4 + dj
                            nc.vector.scalar_tensor_tensor(
                                out=h_sb[:, dci * T:(dci + 1) * T],
                                in0=p_o[dj][:, 0:T],
                                scalar=bout_t[:, dci:dci + 1],
                                in1=h_sb[:, dci * T:(dci + 1) * T],
                                op0=ALU.add, op1=ALU.add,
                            )
                            ln_stats_chunk(p_ss1, dci)

            # ---- final LN with gamma/beta, fp32 apply (in place in h_sb) ----
            with nc.named_scope("final_ln"):
                layer_norm(h_sb, gb=(lnfg_sb, lnfb_sb), p_ss=p_ss1)
                nc.sync.dma_start(
                    out=outT.rearrange("(c p) t -> p c t", p=128),
                    in_=h_sb[:].rearrange("p (c t) -> p c t", t=T),
                )

    nc.compile()
    return nc


_CACHED = {}


def _prep_inputs(inputs_embeds, w_qkv, b_qkv, w_proj, b_proj, w_fc, b_fc,
                 w_out, b_out, ln1_g, ln1_b, ln2_g, ln2_b, lnf_g, lnf_b):
    """Fold LN gamma/beta into weights; permute+scale q/k; cast to fp16."""
    perm = _qk_perm()
    f16 = np.float16
    qs = 0.125 / KV_SCALE
    wqk_l, wv_l, bqk_l, bv_l = [], [], [], []
    wfc_l, bfc_l = [], []
    for l in range(L):
        b_eff = b_qkv[l] + ln1_b[l] @ w_qkv[l]          # [3D]
        w_eff = ln1_g[l][:, None] * w_qkv[l]            # [D, 3D]
        wq = w_eff[:, perm] * qs
        wk = w_eff[:, D + perm] * KV_SCALE
        bq = b_eff[perm] * qs
        bk = b_eff[D + perm] * KV_SCALE
        wqk_l.append(np.concatenate([wq, wk], axis=1).astype(f16))
        wv_l.append((w_eff[:, 2 * D:] * KV_SCALE).astype(f16))
        bqk_l.append(np.concatenate([bq, bk]).reshape(16, 128).T.astype(np.float32))
        # packed later
        bv_l.append((b_eff[2 * D:] * KV_SCALE).reshape(1, D).astype(f16))
        bfc_eff = b_fc[l] + ln2_b[l] @ w_fc[l]
        wfc_l.append((ln2_g[l][:, None] * w_fc[l]).astype(f16))
        bfc_l.append(bfc_eff.reshape(32, 128).T.astype(np.float32))
    bproj_p = b_proj.reshape(L, 8, 128).transpose(0, 2, 1).astype(np.float32)
    bout_p = b_out.reshape(L, 8, 128).transpose(0, 2, 1).astype(np.float32)
    biases_p = np.concatenate(
        [np.stack(bqk_l), bproj_p, np.stack(bfc_l), bout_p], axis=2
    ).astype(np.float32)                                   # [L, 128, 64]
    shared = {
        "wqk": np.stack(wqk_l),
        "wv": np.stack(wv_l),
        "wproj": w_proj.astype(f16),
        "wfc": np.stack(wfc_l),
        "wout": w_out.astype(f16),
        "biases": biases_p,
        "bv": np.stack(bv_l),
        "lnfg": lnf_g.reshape(8, 128).T.astype(np.float32),
        "lnfb": lnf_b.reshape(8, 128).T.astype(np.float32),
    }
    x_flat = np.asarray(inputs_embeds, dtype=np.float32).reshape(B * S, D)
    in_maps = []
    for c in range(N_CORES):
        cos2, ss2 = _rope_tables(c)
        m = dict(shared)
        m["x0T"] = np.ascontiguousarray(x_flat[c * T:(c + 1) * T].T)
        m["cosT"] = cos2
        m["ssT"] = ss2
        m["maskT"] = _causal_mask(c)
        in_maps.append(m)
    return in_maps


def kernel(**inputs):
    inputs = {k: np.asarray(v) for k, v in inputs.items()}
    in_maps = _prep_inputs(
        inputs["inputs_embeds"], inputs["w_qkv"], inputs["b_qkv"],
        inputs["w_proj"], inputs["b_proj"], inputs["w_fc"], inputs["b_fc"],
        inputs["w_out"], inputs["b_out"], inputs["ln1_g"], inputs["ln1_b"],
        inputs["ln2_g"], inputs["ln2_b"], inputs["lnf_g"], inputs["lnf_b"],
    )
    if "nc" not in _CACHED:
        _CACHED["nc"] = build_program()
    res = run_bass_kernel_spmd(_CACHED["nc"], in_maps, list(range(N_CORES)))
    out = np.empty((B * S, D), dtype=np.float32)
    for c in range(N_CORES):
        out[c * T:(c + 1) * T] = res.results[c]["outT"].T
    return out.reshape(B, S, D)


if __name__ == "__main__":
    print("building program...")
    build_program()
    print("built OK")


# revision 27
# speedup vs baseline: 1.0135x; 1.0135x over previous
"""Trainium2 Bass kernel for a 4-layer dense transformer (B=2, S=1024, D=1024, H=16).

Sharding: context-parallel over tokens across 8 cores (256 tokens/core;
cores 0-3 = batch 0, cores 4-7 = batch 1). Per layer, K/V are exchanged
within each 4-core batch group via one AllGather; everything else is local.

On-chip layout: feature-major residual h^T [D, T] so every GEMM consumes
weights in native [in, out] layout as the stationary operand with zero
transposes. Scores are computed as S^T [k, q]; softmax runs over the
partition axis. The softmax denominator comes for free from a scaled
ones-column appended to each head's V block (psum row 64). K/V cross the
wire in fp8 e3m4 scaled by 4 (folded into wk/wv); GEMM operands are
otherwise fp16 (fp32 PSUM accumulate); residual and LN stats stay fp32.
"""

import sys
import os

for _p in ("/opt/trn_rl_repo", "/root/.axon_site/_ro/trn_rl_repo"):
    if os.path.isdir(_p) and _p not in sys.path:
        sys.path.insert(0, _p)

import numpy as np
import concourse.bass as bass
import concourse.bacc as bacc
import concourse.mybir as mybir
import concourse.tile as tile
from concourse.bass_utils import run_bass_kernel_spmd

dt = mybir.dt
AF = mybir.ActivationFunctionType
ALU = mybir.AluOpType

L, B, S, D, H = 4, 2, 1024, 1024, 16
DH = D // H
F = 4 * D
ROPE_BASE = 10000.0
LN_EPS = 1e-5

N_CORES = 8
T = (B * S) // N_CORES            # 256 tokens per core
DC = D // 128                     # 8 feature chunks
HP = H // 2                       # 8 head pairs
GROUPS = [[0, 1, 2, 3], [4, 5, 6, 7]]
RANKS = 4                         # cores per batch group
KV_K = D * T                      # elems of local K^T block
KV_TOT = 2 * KV_K                 # K^T + V per core
KV_SCALE = 4.0                    # fp8 wire scale for K and V (folded in)
VW = DH + 1                       # V block width per head incl. ones col

_SHUF_MASK = [(i + 16) % 32 for i in range(32)]


def _qk_perm():
    """Per-head permutation: [16 even-rows; 16 odd-rows] per 32-row quadrant."""
    perm = np.zeros(D, dtype=np.int64)
    for h in range(H):
        for quad in range(2):
            for j in range(32):
                pair = quad * 16 + (j % 16)
                old_d = 2 * pair + (1 if j >= 16 else 0)
                perm[h * 64 + quad * 32 + j] = h * 64 + old_d
    return perm


def _rope_tables(core):
    """cos [128,2T] fp32 and signed-sin [128,2T] fp16 (tables doubled so a
    pair of feature chunks shares one vector op)."""
    j = core % RANKS
    pos = j * T + np.arange(T, dtype=np.float64)
    inv_freq = 1.0 / (ROPE_BASE ** (np.arange(0, DH, 2, dtype=np.float64) / DH))
    cos128 = np.zeros((128, T), dtype=np.float32)
    ss128 = np.zeros((128, T), dtype=np.float32)
    for p in range(128):
        qq, jj = p // 32, p % 32
        i = (qq % 2) * 16 + (jj % 16)
        ang = pos * inv_freq[i]
        cos128[p] = np.cos(ang)
        ss128[p] = (-np.sin(ang)) if jj < 16 else np.sin(ang)
    cos2 = np.concatenate([cos128, cos128], axis=1)
    ss2 = np.concatenate([ss128, ss128], axis=1).astype(np.float16)
    return cos2, ss2


def _causal_mask(core):
    """maskT [128, DC*T] fp16: mask[p, kc*T + t] = key kc*128+p visible to query t."""
    j = core % RANKS
    q = j * T + np.arange(T)
    m = np.zeros((128, DC * T), dtype=np.float16)
    for kc in range(DC):
        k = kc * 128 + np.arange(128)
        m[:, kc * T:(kc + 1) * T] = (k[:, None] <= q[None, :]).astype(np.float16)
    return m


def build_program():
    nc = bacc.Bacc("TRN2", target_bir_lowering=False, debug=False,
                   num_devices=N_CORES)
    f16, f32, f8 = dt.float16, dt.float32, dt.float8e3

    x0T = nc.dram_tensor("x0T", [D, T], f32, kind="ExternalInput")
    cosT = nc.dram_tensor("cosT", [128, 2 * T], f32, kind="ExternalInput")
    ssT = nc.dram_tensor("ssT", [128, 2 * T], f16, kind="ExternalInput")
    maskT = nc.dram_tensor("maskT", [128, DC * T], f16, kind="ExternalInput")
    wqk = nc.dram_tensor("wqk", [L, D, 2 * D], f16, kind="ExternalInput")
    wv = nc.dram_tensor("wv", [L, D, D], f16, kind="ExternalInput")
    wproj = nc.dram_tensor("wproj", [L, D, D], f16, kind="ExternalInput")
    wfc = nc.dram_tensor("wfc", [L, D, F], f16, kind="ExternalInput")
    wout = nc.dram_tensor("wout", [L, F, D], f16, kind="ExternalInput")
    biases = nc.dram_tensor("biases", [L, 128, 64], f32, kind="ExternalInput")
    bv = nc.dram_tensor("bv", [L, 1, D], f16, kind="ExternalInput")
    lnfg = nc.dram_tensor("lnfg", [128, 8], f32, kind="ExternalInput")
    lnfb = nc.dram_tensor("lnfb", [128, 8], f32, kind="ExternalInput")
    outT = nc.dram_tensor("outT", [D, T], f32, kind="ExternalOutput")

    from contextlib import ExitStack
    with ExitStack() as _es:
        tc = _es.enter_context(tile.TileContext(nc))
        pp = _es.enter_context(tc.tile_pool(name="persist", bufs=1))
        wqk_pool = _es.enter_context(tc.tile_pool(name="wqk", bufs=1))
        wv_pool = _es.enter_context(tc.tile_pool(name="wv", bufs=1))
        wpr_pool = _es.enter_context(tc.tile_pool(name="wpr", bufs=1))
        wfc_pool = _es.enter_context(tc.tile_pool(name="wfc", bufs=3))
        wout_pool = _es.enter_context(tc.tile_pool(name="wout", bufs=2))
        bias_pool = _es.enter_context(tc.tile_pool(name="bias", bufs=2))
        xh_pool = _es.enter_context(tc.tile_pool(name="xh", bufs=1))
        t16_pool = _es.enter_context(tc.tile_pool(name="ln16", bufs=2))
        rope_pool = _es.enter_context(tc.tile_pool(name="rope", bufs=2))
        t32_pool = _es.enter_context(tc.tile_pool(name="tmp32", bufs=2))
        probs_pool = _es.enter_context(tc.tile_pool(name="probs", bufs=4))
        stat_pool = _es.enter_context(tc.tile_pool(name="stat", bufs=1))
        bc_pool = _es.enter_context(tc.tile_pool(name="bcast", bufs=2))
        ps_small = _es.enter_context(tc.tile_pool(name="ps_small", bufs=1, space="PSUM"))
        ps_bank = _es.enter_context(tc.tile_pool(name="ps_bank", bufs=5, space="PSUM"))
        ps_at = _es.enter_context(tc.tile_pool(name="ps_at", bufs=2, space="PSUM"))
        dram = _es.enter_context(tc.tile_pool(name="dram", bufs=1, space="DRAM"))
        if True:
            h_sb = pp.tile([128, DC * T], f32)
            cos_sb = pp.tile([128, 2 * T], f32)
            ss_sb = pp.tile([128, 2 * T], f16)
            mask_sb = pp.tile([128, DC * T], f16)
            Q_sb = pp.tile([128, HP * T], f16)
            Kl_sb = pp.tile([128, HP * T], f8)
            Vl_sb = pp.tile([128, 2 * D], f8)
            K_sb = pp.tile([128, HP * S], f8)
            V_sb = pp.tile([128, DC * H * VW], f8)
            attn_sb = pp.tile([128, DC * T], f16)
            h1_sb = pp.tile([128, (F // 128) * T], f16)  # [128, 8192]
            ones_c = pp.tile([128, 1], f16)
            ones_r = pp.tile([1, 128], f16)
            eps_c = pp.tile([1, 1], f32)
            lnfg_sb = pp.tile([128, 8], f32)
            lnfb_sb = pp.tile([128, 8], f32)

            kvloc = dram.tile([KV_TOT], f8)
            kvag = dram.tile([RANKS * KV_TOT], f8)

            nc.vector.memset(ones_c[:], 1.0)
            nc.vector.memset(ones_r[:], 1.0)
            nc.vector.memset(eps_c[:], LN_EPS)
            # ones columns (col DH of each head block) give the softmax
            # denominator; the V unpack DMAs only ever write cols 0..DH-1.
            nc.vector.memset(V_sb[:], KV_SCALE)
            nc.sync.dma_start(out=cos_sb[:], in_=cosT[:])
            nc.sync.dma_start(out=ss_sb[:], in_=ssT[:])
            nc.sync.dma_start(out=mask_sb[:], in_=maskT[:])
            nc.sync.dma_start(out=lnfg_sb[:], in_=lnfg[:])
            nc.sync.dma_start(out=lnfb_sb[:], in_=lnfb[:])
            for ci in range(DC):
                nc.sync.dma_start(
                    out=h_sb[:, ci * T:(ci + 1) * T],
                    in_=x0T[ci * 128:(ci + 1) * 128, :],
                )

            def load_qkv_weights(l):
                wqk_t = wqk_pool.tile([128, DC * 2 * D], f16, tag="wqk")
                wv_t = wv_pool.tile([128, DC * D], f16, tag="wv")
                nc.sync.dma_start(
                    out=wqk_t[:].rearrange("p (c n) -> p c n", c=DC),
                    in_=wqk[l].rearrange("(c p) n -> p c n", p=128),
                )
                nc.sync.dma_start(
                    out=wv_t[:].rearrange("p (c n) -> p c n", c=DC),
                    in_=wv[l].rearrange("(c p) n -> p c n", p=128),
                )
                return wqk_t, wv_t

            def ln_stats_chunk(p_ss, ci):
                """Accumulate sum(h) | sum(h^2) for chunk ci into p_ss."""
                hc = h_sb[:, ci * T:(ci + 1) * T]
                hsq = t16_pool.tile([128, 2 * T], f16, tag="hsq")
                nc.vector.tensor_copy(hsq[:, 0:T], hc)
                nc.scalar.activation(hsq[:, T:2 * T], hc, AF.Square)
                nc.tensor.matmul(p_ss[:], ones_c[:], hsq[:],
                                 start=(ci == 0), stop=(ci == DC - 1))

            def layer_norm(xhat, gb=None, p_ss=None):
                """h_sb (f32) -> xhat (f16 [128, DC*T]) normalized. If gb is
                given, apply per-feature gamma/beta (final LN, f32 out). If
                p_ss is given, the stats were already accumulated inline."""
                if p_ss is None:
                    p_ss = ps_small.tile([1, 2 * T], f32, tag="ps_small")
                    for ci in range(DC):
                        ln_stats_chunk(p_ss, ci)
                m = stat_pool.tile([1, T], f32, tag="st_m")
                msq = stat_pool.tile([1, T], f32, tag="st_msq")
                var = stat_pool.tile([1, T], f32, tag="st_var")
                rstd = stat_pool.tile([1, T], f32, tag="st_rstd")
                mr = stat_pool.tile([1, T], f32, tag="st_mr")
                nc.vector.tensor_scalar_mul(m[:], p_ss[:, 0:T], 1.0 / D)
                nc.vector.tensor_scalar_mul(msq[:], p_ss[:, T:2 * T], 1.0 / D)
                nc.vector.tensor_tensor(out=var[:], in0=m[:], in1=m[:], op=ALU.mult)
                nc.vector.tensor_sub(var[:], msq[:], var[:])
                # rstd = exp(-0.5 * ln(var + eps)) — keeps ACT on the ln/exp table
                nc.scalar.activation(var[:], var[:], AF.Ln, bias=eps_c[:])
                nc.vector.tensor_scalar_mul(var[:], var[:], -0.5)
                nc.scalar.activation(rstd[:], var[:], AF.Exp)
                nc.vector.tensor_tensor(out=mr[:], in0=m[:], in1=rstd[:], op=ALU.mult)
                rstd_b = bc_pool.tile([128, T], f32, tag="rstd_b")
                mr_b = bc_pool.tile([128, T], f32, tag="mr_b")
                nc.gpsimd.partition_broadcast(rstd_b[:], rstd[:])
                nc.gpsimd.partition_broadcast(mr_b[:], mr[:])
                for ci in range(DC):
                    hc = h_sb[:, ci * T:(ci + 1) * T]
                    u = t32_pool.tile([128, T], f32, tag="ln_u")
                    nc.vector.tensor_tensor(out=u[:], in0=hc, in1=rstd_b[:],
                                            op=ALU.mult)
                    if gb is None:
                        nc.vector.tensor_tensor(out=xhat[:, ci * T:(ci + 1) * T],
                                                in0=u[:], in1=mr_b[:],
                                                op=ALU.subtract)
                    else:
                        g_sb, b_sb = gb
                        z = t32_pool.tile([128, T], f32, tag="ln_z")
                        nc.vector.tensor_tensor(out=z[:], in0=u[:], in1=mr_b[:],
                                                op=ALU.subtract)
                        nc.vector.tensor_scalar(
                            out=xhat[:, ci * T:(ci + 1) * T], in0=z[:],
                            scalar1=g_sb[:, ci:ci + 1], scalar2=b_sb[:, ci:ci + 1],
                            op0=ALU.mult, op1=ALU.add,
                        )

            def rope_pair(p_qk, dest, pi, bqk_t, bias_off):
                """p_qk [128,2T] psum holding chunk pair (2pi, 2pi+1); write
                RoPE'd pair into dest[:, 2pi*T:(2pi+2)*T]."""
                qtmp = rope_pool.tile([128, 2 * T], f16, tag="rope_q")
                ctmp = rope_pool.tile([128, 2 * T], f16, tag="rope_c")
                stmp = rope_pool.tile([128, 2 * T], f16, tag="rope_s")
                dtmp = rope_pool.tile([128, 2 * T], f16, tag="rope_d")
                for half in range(2):
                    bcol = bqk_t[:, bias_off + 2 * pi + half:bias_off + 2 * pi + half + 1]
                    sl = slice(half * T, (half + 1) * T)
                    nc.vector.tensor_scalar_add(qtmp[:, sl], p_qk[:, sl], bcol)
                    nc.vector.scalar_tensor_tensor(
                        out=ctmp[:, sl], in0=p_qk[:, sl], scalar=bcol,
                        in1=cos_sb[:, sl], op0=ALU.add, op1=ALU.mult,
                    )
                nc.vector.stream_shuffle(stmp[:], qtmp[:], _SHUF_MASK)
                nc.vector.tensor_tensor(out=dtmp[:], in0=stmp[:], in1=ss_sb[:],
                                        op=ALU.mult)
                nc.vector.tensor_tensor(
                    out=dest[:, 2 * pi * T:(2 * pi + 2) * T],
                    in0=ctmp[:], in1=dtmp[:], op=ALU.add,
                )

            def load_biases(l):
                bias_t = bias_pool.tile([128, 64], f32, tag="biases")
                bv_t = bias_pool.tile([1, D], f16, tag="bv")
                nc.sync.dma_start(out=bias_t[:], in_=biases[l])
                nc.sync.dma_start(out=bv_t[:], in_=bv[l])
                return bias_t, bv_t

            wqk_t, wv_t = load_qkv_weights(0)
            bias_tiles = {0: load_biases(0)}

            for l in range(L):
                # ---- per-layer bias tiles (prefetched) + proj weights ----
                bias_t, bv_t = bias_tiles[l]
                bqk_t = bias_t[:, 0:16]
                bproj_t = bias_t[:, 16:24]
                bfc_t = bias_t[:, 24:56]
                bout_t = bias_t[:, 56:64]
                wproj_t = wpr_pool.tile([128, DC * D], f16, tag="wproj")
                nc.sync.dma_start(
                    out=wproj_t[:].rearrange("p (c n) -> p c n", c=DC),
                    in_=wproj[l].rearrange("(c p) n -> p c n", p=128),
                )

                # ---- LN1 ----
                with nc.named_scope("ln1"):
                    xhat = xh_pool.tile([128, DC * T], f16, tag="xhat")
                    layer_norm(xhat, p_ss=(None if l == 0 else p_ss1))

                # ---- K projection + RoPE (first, so the gather launches early)
                with nc.named_scope("kv_proj"):
                    for pi in range(4):      # k chunk pairs
                        p_qk = ps_bank.tile([128, 2 * T], f32, tag="ps_bank")
                        for half in range(2):
                            fci = HP + 2 * pi + half
                            for dci in range(DC):
                                nc.tensor.matmul(
                                    p_qk[:, half * T:(half + 1) * T],
                                    wqk_t[:, dci * 2 * D + fci * 128:
                                      dci * 2 * D + (fci + 1) * 128],
                                    xhat[:, dci * T:(dci + 1) * T],
                                    start=(dci == 0), stop=(dci == DC - 1),
                                )
                        rope_pair(p_qk, Kl_sb, pi, bqk_t, HP)
                    # v: token-major [T, D] via lhsT = xhat slices
                    for tci in range(2):
                        for fh in range(2):
                            p_v = ps_bank.tile([128, 512], f32, tag="ps_bank")
                            for dci in range(DC):
                                nc.tensor.matmul(
                                    p_v[:],
                                    xhat[:, dci * T + tci * 128: dci * T + (tci + 1) * 128],
                                    wv_t[:, dci * D + fh * 512:
                                         dci * D + (fh + 1) * 512],
                                    start=(dci == 0), stop=False,
                                )
                            nc.tensor.matmul(
                                p_v[:], ones_r[:], bv_t[:, fh * 512:(fh + 1) * 512],
                                start=False, stop=True,
                            )
                            nc.vector.tensor_copy(
                                Vl_sb[:, tci * D + fh * 512: tci * D + (fh + 1) * 512],
                                p_v[:],
                            )

                # ---- stage K^T,V to DRAM; AllGather within batch group ----
                with nc.named_scope("kv_gather"):
                    for pi in range(4):
                        nc.sync.dma_start(
                            out=kvloc[2 * pi * T * 128:(2 * pi + 2) * T * 128]
                                .rearrange("(c p t) -> p c t", p=128, t=T),
                            in_=Kl_sb[:, 2 * pi * T:(2 * pi + 2) * T]
                                .rearrange("p (c t) -> p c t", t=T),
                        )
                    nc.sync.dma_start(
                        out=kvloc[KV_K:KV_TOT].rearrange(
                            "(c p f) -> p c f", p=128, f=D),
                        in_=Vl_sb[:].rearrange("p (c f) -> p c f", f=D),
                    )
                    nc.gpsimd.collective_compute(
                        "AllGather",
                        ALU.bypass,
                        ins=[kvloc.opt()],
                        outs=[kvag.opt()],
                        replica_groups=GROUPS,
                    )

                def issue_wfc(g):
                    t = wfc_pool.tile([128, DC * 512], f16, tag="wfc")
                    nc.sync.dma_start(
                        out=t[:].rearrange("p (c n) -> p c n", c=DC),
                        in_=wfc[l].rearrange("(c p) n -> p c n", p=128)
                            [:, :, g * 512:(g + 1) * 512],
                    )
                    return t

                def issue_wout(i):
                    half, fcg = i // 4, i % 4
                    t = wout_pool.tile([128, 8 * 512], f16, tag="wout")
                    nc.sync.dma_start(
                        out=t[:].rearrange("p (c n) -> p c n", c=8),
                        in_=wout[l].rearrange("(c p) n -> p c n", p=128)
                            [:, fcg * 8:(fcg + 1) * 8,
                             half * 512:(half + 1) * 512],
                    )
                    return t

                wfc_ts = {g: issue_wfc(g) for g in range(3)}
                wout_ts = {i: issue_wout(i) for i in range(2)}

                # ---- Q projection + RoPE (overlaps the gather) ----
                with nc.named_scope("q_proj"):
                    for pi in range(4):      # q chunk pairs
                        p_qk = ps_bank.tile([128, 2 * T], f32, tag="ps_bank")
                        for half in range(2):
                            fci = 2 * pi + half
                            for dci in range(DC):
                                nc.tensor.matmul(
                                    p_qk[:, half * T:(half + 1) * T],
                                    wqk_t[:, dci * 2 * D + fci * 128:
                                      dci * 2 * D + (fci + 1) * 128],
                                    xhat[:, dci * T:(dci + 1) * T],
                                    start=(dci == 0), stop=(dci == DC - 1),
                                )
                        rope_pair(p_qk, Q_sb, pi, bqk_t, 0)

                # ---- unpack gathered K/V ----
                with nc.named_scope("kv_unpack"):
                    for rr in range(RANKS):
                        base = rr * KV_TOT
                        nc.sync.dma_start(
                            out=K_sb[:, rr * HP * T:(rr + 1) * HP * T].rearrange(
                                "p (c t) -> p c t", t=T),
                            in_=kvag[base:base + KV_K].rearrange(
                                "(c p t) -> p c t", p=128, t=T),
                        )
                        for tci in range(2):
                            vbase = base + KV_K + tci * D * 128
                            nc.sync.dma_start(
                                out=V_sb[:].rearrange(
                                    "p (c h x) -> p c h x", h=H, x=VW
                                )[:, 2 * rr + tci, :, 0:DH],
                                in_=kvag[vbase:vbase + D * 128].rearrange(
                                    "(p h d) -> p h d", p=128, d=DH),
                            )

                # ---- attention ----
                with nc.named_scope("attn"):
                    for hp in range(HP):
                        p_at = ps_at.tile([VW, 2 * T], f32, tag="ps_at")
                        for hh in range(2):
                            bp = hh * 64
                            hglob = 2 * hp + hh
                            for kp in range(DC // 2):
                                p_s = ps_bank.tile([128, 2 * T], f32, tag="ps_bank")
                                for half in range(2):
                                    kc = 2 * kp + half
                                    koff = ((kc // 2) * HP * T + hp * T
                                            + (kc % 2) * 128)
                                    nc.tensor.matmul(
                                        p_s[:, half * T:(half + 1) * T],
                                        K_sb[bp:bp + 64, koff:koff + 128],
                                        Q_sb[bp:bp + 64, hp * T:(hp + 1) * T],
                                        start=True, stop=True,
                                    )
                                probs = probs_pool.tile([128, 2 * T], f16, tag="probs")
                                nc.scalar.activation(probs[:], p_s[:], AF.Exp)
                                nc.vector.tensor_tensor(
                                    out=probs[:], in0=probs[:],
                                    in1=mask_sb[:, 2 * kp * T:(2 * kp + 2) * T],
                                    op=ALU.mult,
                                )
                                for half in range(2):
                                    kc = 2 * kp + half
                                    nc.tensor.matmul(
                                        p_at[0:VW, hh * T:(hh + 1) * T],
                                        V_sb[:, kc * H * VW + hglob * VW:
                                             kc * H * VW + (hglob + 1) * VW],
                                        probs[:, half * T:(half + 1) * T],
                                        start=(kc == 0), stop=(kc == DC - 1),
                                    )
                        for hh in range(2):
                            recip = stat_pool.tile([1, T], f32, tag="recip")
                            nc.vector.reciprocal(
                                recip[:], p_at[DH:DH + 1, hh * T:(hh + 1) * T])
                            rb = bc_pool.tile([64, T], f32, tag="rb")
                            nc.gpsimd.partition_broadcast(rb[:], recip[:])
                            nc.vector.tensor_tensor(
                                out=attn_sb[hh * 64:(hh + 1) * 64,
                                            hp * T:(hp + 1) * T],
                                in0=p_at[0:64, hh * T:(hh + 1) * T],
                                in1=rb[:],
                                op=ALU.mult,
                            )

                # ---- attention out-proj + residual ----
                with nc.named_scope("proj"):
                    p_ss2 = ps_small.tile([1, 2 * T], f32, tag="ps_small")
                    for half in range(2):
                        p_pr = [ps_bank.tile([128, 2 * T], f32, tag="ps_bank",
                                             name=f"p_pr{dj}") for dj in range(2)]
                        for dj in range(4):
                            for cin in range(DC):
                                off = cin * D + half * 512 + dj * 128
                                nc.tensor.matmul(
                                    p_pr[dj // 2][:, (dj % 2) * T:(dj % 2 + 1) * T],
                                    wproj_t[:, off:off + 128],
                                    attn_sb[:, cin * T:(cin + 1) * T],
                                    start=(cin == 0), stop=(cin == DC - 1),
                                )
                        for dj in range(4):
                            dci = half * 4 + dj
                            nc.vector.scalar_tensor_tensor(
                                out=h_sb[:, dci * T:(dci + 1) * T],
                                in0=p_pr[dj // 2][:, (dj % 2) * T:(dj % 2 + 1) * T],
                                scalar=bproj_t[:, dci:dci + 1],
                                in1=h_sb[:, dci * T:(dci + 1) * T],
                                op0=ALU.add, op1=ALU.add,
                            )
                            ln_stats_chunk(p_ss2, dci)

                # ---- LN2 ----
                with nc.named_scope("ln2"):
                    xhat2 = xh_pool.tile([128, DC * T], f16, tag="xhat")
                    layer_norm(xhat2, p_ss=p_ss2)

                # prefetch next layer's qkv weights + biases (overlaps the FFN)
                if l + 1 < L:
                    wqk_t, wv_t = load_qkv_weights(l + 1)
                    bias_tiles[l + 1] = load_biases(l + 1)

                # ---- FFN: fc + gelu -> h1, then out-proj + residual ----
                with nc.named_scope("fc"):
                    for g in range(F // 512):          # 8 groups of 4 output chunks
                        p_fc = [ps_bank.tile([128, 2 * T], f32, tag="ps_bank",
                                             name=f"p_fc{fj}") for fj in range(2)]
                        wfc_t = wfc_ts[g]
                        if g + 3 < F // 512:
                            wfc_ts[g + 3] = issue_wfc(g + 3)
                        for fj in range(4):
                            for dci in range(DC):
                                nc.tensor.matmul(
                                    p_fc[fj // 2][:, (fj % 2) * T:(fj % 2 + 1) * T],
                                    wfc_t[:, dci * 512 + fj * 128:
                                          dci * 512 + (fj + 1) * 128],
                                    xhat2[:, dci * T:(dci + 1) * T],
                                    start=(dci == 0), stop=(dci == DC - 1),
                                )
                        for fj in range(4):
                            fci = g * 4 + fj
                            nc.scalar.activation(
                                h1_sb[:, fci * T:(fci + 1) * T],
                                p_fc[fj // 2][:, (fj % 2) * T:(fj % 2 + 1) * T],
                                AF.Gelu_apprx_tanh,
                                bias=bfc_t[:, fci:fci + 1],
                            )
                with nc.named_scope("ffn_out"):
                    warm = stat_pool.tile([1, 1], f32, tag="warm")
                    nc.scalar.activation(warm[:], eps_c[:], AF.Ln)
                    p_ss1 = ps_small.tile([1, 2 * T], f32, tag="ps_small")
                    for half in range(2):
                        p_o = [ps_bank.tile([128, 2 * T], f32, tag="ps_bank",
                                            name=f"p_o{dj}") for dj in range(4)]
                        for fcg in range(4):       # 32 contraction chunks in 4 groups
                            i = half * 4 + fcg
                            wout_t = wout_ts[i]
                            if i + 2 < 8:
                                wout_ts[i + 2] = issue_wout(i + 2)
                            for fcl in range(8):
                                fci = fcg * 8 + fcl
                                for dj in range(4):
                                    nc.tensor.matmul(
                                        p_o[dj][:, 0:T],
                                        wout_t[:, fcl * 512 + dj * 128:
                                               fcl * 512 + (dj + 1) * 128],
                                        h1_sb[:, fci * T:(fci + 1) * T],
                                        start=(fci == 0), stop=(fci == F // 128 - 1),
                                    )
                        for dj in range(4):
                            dci = half * 4 + dj
                            nc.vector.scalar_tensor_tensor(
                                out=h_sb[:, dci * T:(dci + 1) * T],
                                in0=p_o[dj][:, 0:T],
                                scalar=bout_t[:, dci:dci + 1],
                                in1=h_sb[:, dci * T:(dci + 1) * T],
                                op0=ALU.add, op1=ALU.add,
                            )
                            ln_stats_chunk(p_ss1, dci)

            # ---- final LN with gamma/beta, fp32 apply (in place in h_sb) ----
            with nc.named_scope("final_ln"):
                layer_norm(h_sb, gb=(lnfg_sb, lnfb_sb), p_ss=p_ss1)
                nc.sync.dma_start(
                    out=outT.rearrange("(c p) t -> p c t", p=128),
                    in_=h_sb[:].rearrange("p (c t) -> p c t", t=T),
                )

    nc.compile()
    return nc


_CACHED = {}


def _prep_inputs(inputs_embeds, w_qkv, b_qkv, w_proj, b_proj, w_fc, b_fc,
                 w_out, b_out, ln1_g, ln1_b, ln2_g, ln2_b, lnf_g, lnf_b):
    """Fold LN gamma/beta into weights; permute+scale q/k; cast to fp16."""
    perm = _qk_perm()
    f16 = np.float16
    qs = 0.125 / KV_SCALE
    wqk_l, wv_l, bqk_l, bv_l = [], [], [], []
    wfc_l, bfc_l = [], []
    for l in range(L):
        b_eff = b_qkv[l] + ln1_b[l] @ w_qkv[l]          # [3D]
        w_eff = ln1_g[l][:, None] * w_qkv[l]            # [D, 3D]
        wq = w_eff[:, perm] * qs
        wk = w_eff[:, D + perm] * KV_SCALE
        bq = b_eff[perm] * qs
        bk = b_eff[D + perm] * KV_SCALE
        wqk_l.append(np.concatenate([wq, wk], axis=1).astype(f16))
        wv_l.append((w_eff[:, 2 * D:] * KV_SCALE).astype(f16))
        bqk_l.append(np.concatenate([bq, bk]).reshape(16, 128).T.astype(np.float32))
        # packed later
        bv_l.append((b_eff[2 * D:] * KV_SCALE).reshape(1, D).astype(f16))
        bfc_eff = b_fc[l] + ln2_b[l] @ w_fc[l]
        wfc_l.append((ln2_g[l][:, None] * w_fc[l]).astype(f16))
        bfc_l.append(bfc_eff.reshape(32, 128).T.astype(np.float32))
    bproj_p = b_proj.reshape(L, 8, 128).transpose(0, 2, 1).astype(np.float32)
    bout_p = b_out.reshape(L, 8, 128).transpose(0, 2, 1).astype(np.float32)
    biases_p = np.concatenate(
        [np.stack(bqk_l), bproj_p, np.stack(bfc_l), bout_p], axis=2
    ).astype(np.float32)                                   # [L, 128, 64]
    shared = {
        "wqk": np.stack(wqk_l),
        "wv": np.stack(wv_l),
        "wproj": w_proj.astype(f16),
        "wfc": np.stack(wfc_l),
        "wout": w_out.astype(f16),
        "biases": biases_p,
        "bv": np.stack(bv_l),
        "lnfg": lnf_g.reshape(8, 128).T.astype(np.float32),
        "lnfb": lnf_b.reshape(8, 128).T.astype(np.float32),
    }
    x_flat = np.asarray(inputs_embeds, dtype=np.float32).reshape(B * S, D)
    in_maps = []
    for c in range(N_CORES):
        cos2, ss2 = _rope_tables(c)
        m = dict(shared)
        m["x0T"] = np.ascontiguousarray(x_flat[c * T:(c + 1) * T].T)
        m["cosT"] = cos2
        m["ssT"] = ss2
        m["maskT"] = _causal_mask(c)
        in_maps.append(m)
    return in_maps


def kernel(**inputs):
    inputs = {k: np.asarray(v) for k, v in inputs.items()}
    in_maps = _prep_inputs(
        inputs["inputs_embeds"], inputs["w_qkv"], inputs["b_qkv"],
        inputs["w_proj"], inputs["b_proj"], inputs["w_fc"], inputs["b_fc"],
        inputs["w_out"], inputs["b_out"], inputs["ln1_g"], inputs["ln1_b"],
        inputs["ln2_g"], inputs["ln2_b"], inputs["lnf_g"], inputs["lnf_b"],
    )
    if "nc" not in _CACHED:
        _CACHED["nc"] = build_program()
    res = run_bass_kernel_spmd(_CACHED["nc"], in_maps, list(range(N_CORES)))
    out = np.empty((B * S, D), dtype=np.float32)
    for c in range(N_CORES):
        out[c * T:(c + 1) * T] = res.results[c]["outT"].T
    return out.reshape(B, S, D)


if __name__ == "__main__":
    print("building program...")
    build_program()
    print("built OK")


# revision 28
# speedup vs baseline: 1.0301x; 1.0164x over previous
"""Trainium2 Bass kernel for a 4-layer dense transformer (B=2, S=1024, D=1024, H=16).

Sharding: context-parallel over tokens across 8 cores (256 tokens/core;
cores 0-3 = batch 0, cores 4-7 = batch 1). Per layer, K/V are exchanged
within each 4-core batch group via one AllGather; everything else is local.

On-chip layout: feature-major residual h^T [D, T] so every GEMM consumes
weights in native [in, out] layout as the stationary operand with zero
transposes. Scores are computed as S^T [k, q]; softmax runs over the
partition axis. The softmax denominator comes for free from a scaled
ones-column appended to each head's V block (psum row 64). K/V cross the
wire in fp8 e3m4 scaled by 4 (folded into wk/wv); GEMM operands are
otherwise fp16 (fp32 PSUM accumulate); residual and LN stats stay fp32.
"""

import sys
import os

for _p in ("/opt/trn_rl_repo", "/root/.axon_site/_ro/trn_rl_repo"):
    if os.path.isdir(_p) and _p not in sys.path:
        sys.path.insert(0, _p)

import numpy as np
import concourse.bass as bass
import concourse.bacc as bacc
import concourse.mybir as mybir
import concourse.tile as tile
from concourse.bass_utils import run_bass_kernel_spmd

dt = mybir.dt
AF = mybir.ActivationFunctionType
ALU = mybir.AluOpType

L, B, S, D, H = 4, 2, 1024, 1024, 16
DH = D // H
F = 4 * D
ROPE_BASE = 10000.0
LN_EPS = 1e-5

N_CORES = 8
T = (B * S) // N_CORES            # 256 tokens per core
DC = D // 128                     # 8 feature chunks
HP = H // 2                       # 8 head pairs
GROUPS = [[0, 1, 2, 3], [4, 5, 6, 7]]
RANKS = 4                         # cores per batch group
KV_K = D * T                      # elems of local K^T block
KV_TOT = 2 * KV_K                 # K^T + V per core
KV_SCALE = 4.0                    # fp8 wire scale for K and V (folded in)
VW = DH + 1                       # V block width per head incl. ones col

_SHUF_MASK = [(i + 16) % 32 for i in range(32)]


def _qk_perm():
    """Per-head permutation: [16 even-rows; 16 odd-rows] per 32-row quadrant."""
    perm = np.zeros(D, dtype=np.int64)
    for h in range(H):
        for quad in range(2):
            for j in range(32):
                pair = quad * 16 + (j % 16)
                old_d = 2 * pair + (1 if j >= 16 else 0)
                perm[h * 64 + quad * 32 + j] = h * 64 + old_d
    return perm


def _rope_tables(core):
    """cos [128,2T] fp32 and signed-sin [128,2T] fp16 (tables doubled so a
    pair of feature chunks shares one vector op)."""
    j = core % RANKS
    pos = j * T + np.arange(T, dtype=np.float64)
    inv_freq = 1.0 / (ROPE_BASE ** (np.arange(0, DH, 2, dtype=np.float64) / DH))
    cos128 = np.zeros((128, T), dtype=np.float32)
    ss128 = np.zeros((128, T), dtype=np.float32)
    for p in range(128):
        qq, jj = p // 32, p % 32
        i = (qq % 2) * 16 + (jj % 16)
        ang = pos * inv_freq[i]
        cos128[p] = np.cos(ang)
        ss128[p] = (-np.sin(ang)) if jj < 16 else np.sin(ang)
    cos2 = np.concatenate([cos128, cos128], axis=1)
    ss2 = np.concatenate([ss128, ss128], axis=1).astype(np.float16)
    return cos2, ss2


def _causal_mask(core):
    """maskT [128, DC*T] fp16: mask[p, kc*T + t] = key kc*128+p visible to query t."""
    j = core % RANKS
    q = j * T + np.arange(T)
    m = np.zeros((128, DC * T), dtype=np.float16)
    for kc in range(DC):
        k = kc * 128 + np.arange(128)
        m[:, kc * T:(kc + 1) * T] = (k[:, None] <= q[None, :]).astype(np.float16)
    return m


def build_program():
    nc = bacc.Bacc("TRN2", target_bir_lowering=False, debug=False,
                   num_devices=N_CORES)
    f16, f32, f8 = dt.float16, dt.float32, dt.float8e3

    x0T = nc.dram_tensor("x0T", [D, T], f32, kind="ExternalInput")
    cosT = nc.dram_tensor("cosT", [128, 2 * T], f32, kind="ExternalInput")
    ssT = nc.dram_tensor("ssT", [128, 2 * T], f16, kind="ExternalInput")
    maskT = nc.dram_tensor("maskT", [128, DC * T], f16, kind="ExternalInput")
    wqk = nc.dram_tensor("wqk", [L, D, 2 * D], f16, kind="ExternalInput")
    wv = nc.dram_tensor("wv", [L, D, D], f16, kind="ExternalInput")
    wproj = nc.dram_tensor("wproj", [L, D, D], f16, kind="ExternalInput")
    wfc = nc.dram_tensor("wfc", [L, D, F], f16, kind="ExternalInput")
    wout = nc.dram_tensor("wout", [L, F, D], f16, kind="ExternalInput")
    biases = nc.dram_tensor("biases", [L, 128, 64], f32, kind="ExternalInput")
    bv = nc.dram_tensor("bv", [L, 1, D], f16, kind="ExternalInput")
    lnfg = nc.dram_tensor("lnfg", [128, 8], f32, kind="ExternalInput")
    lnfb = nc.dram_tensor("lnfb", [128, 8], f32, kind="ExternalInput")
    outT = nc.dram_tensor("outT", [D, T], f32, kind="ExternalOutput")

    from contextlib import ExitStack
    with ExitStack() as _es:
        tc = _es.enter_context(tile.TileContext(nc))
        pp = _es.enter_context(tc.tile_pool(name="persist", bufs=1))
        wqk_pool = _es.enter_context(tc.tile_pool(name="wqk", bufs=1))
        wv_pool = _es.enter_context(tc.tile_pool(name="wv", bufs=1))
        wpr_pool = _es.enter_context(tc.tile_pool(name="wpr", bufs=1))
        wfc_pool = _es.enter_context(tc.tile_pool(name="wfc", bufs=3))
        wout_pool = _es.enter_context(tc.tile_pool(name="wout", bufs=2))
        bias_pool = _es.enter_context(tc.tile_pool(name="bias", bufs=2))
        xh_pool = _es.enter_context(tc.tile_pool(name="xh", bufs=1))
        t16_pool = _es.enter_context(tc.tile_pool(name="ln16", bufs=2))
        rope_pool = _es.enter_context(tc.tile_pool(name="rope", bufs=2))
        t32_pool = _es.enter_context(tc.tile_pool(name="tmp32", bufs=2))
        probs_pool = _es.enter_context(tc.tile_pool(name="probs", bufs=4))
        stat_pool = _es.enter_context(tc.tile_pool(name="stat", bufs=1))
        bc_pool = _es.enter_context(tc.tile_pool(name="bcast", bufs=2))
        ps_small = _es.enter_context(tc.tile_pool(name="ps_small", bufs=1, space="PSUM"))
        ps_bank = _es.enter_context(tc.tile_pool(name="ps_bank", bufs=5, space="PSUM"))
        ps_at = _es.enter_context(tc.tile_pool(name="ps_at", bufs=2, space="PSUM"))
        dram = _es.enter_context(tc.tile_pool(name="dram", bufs=1, space="DRAM"))
        if True:
            h_sb = pp.tile([128, DC * T], f32)
            cos_sb = pp.tile([128, 2 * T], f32)
            ss_sb = pp.tile([128, 2 * T], f16)
            mask_sb = pp.tile([128, DC * T], f16)
            Q_sb = pp.tile([128, HP * T], f16)
            Kl_sb = pp.tile([128, HP * T], f8)
            Vl_sb = pp.tile([128, 2 * D], f8)
            K_sb = pp.tile([128, HP * S], f8)
            V_sb = pp.tile([128, DC * H * VW], f8)
            attn_sb = pp.tile([128, DC * T], f16)
            h1_sb = pp.tile([128, (F // 128) * T], f16)  # [128, 8192]
            ones_c = pp.tile([128, 1], f16)
            ones_r = pp.tile([1, 128], f16)
            eps_c = pp.tile([1, 1], f32)
            lnfg_sb = pp.tile([128, 8], f32)
            lnfb_sb = pp.tile([128, 8], f32)

            kvloc = dram.tile([KV_TOT], f8)
            kvag = dram.tile([RANKS * KV_TOT], f8)

            nc.vector.memset(ones_c[:], 1.0)
            nc.vector.memset(ones_r[:], 1.0)
            nc.vector.memset(eps_c[:], LN_EPS)
            # ones columns (col DH of each head block) give the softmax
            # denominator; the V unpack DMAs only ever write cols 0..DH-1.
            nc.vector.memset(V_sb[:], KV_SCALE)
            nc.sync.dma_start(out=cos_sb[:], in_=cosT[:])
            nc.sync.dma_start(out=ss_sb[:], in_=ssT[:])
            nc.sync.dma_start(out=mask_sb[:], in_=maskT[:])
            nc.sync.dma_start(out=lnfg_sb[:], in_=lnfg[:])
            nc.sync.dma_start(out=lnfb_sb[:], in_=lnfb[:])
            for ci in range(DC):
                nc.sync.dma_start(
                    out=h_sb[:, ci * T:(ci + 1) * T],
                    in_=x0T[ci * 128:(ci + 1) * 128, :],
                )

            def load_qkv_weights(l):
                wqk_t = wqk_pool.tile([128, DC * 2 * D], f16, tag="wqk")
                wv_t = wv_pool.tile([128, DC * D], f16, tag="wv")
                nc.sync.dma_start(
                    out=wqk_t[:].rearrange("p (c n) -> p c n", c=DC),
                    in_=wqk[l].rearrange("(c p) n -> p c n", p=128),
                )
                nc.sync.dma_start(
                    out=wv_t[:].rearrange("p (c n) -> p c n", c=DC),
                    in_=wv[l].rearrange("(c p) n -> p c n", p=128),
                )
                return wqk_t, wv_t

            def ln_stats_chunk(p_ss, ci):
                """Accumulate sum(h) | sum(h^2) for chunk ci into p_ss."""
                hc = h_sb[:, ci * T:(ci + 1) * T]
                hsq = t16_pool.tile([128, 2 * T], f16, tag="hsq")
                nc.vector.tensor_copy(hsq[:, 0:T], hc)
                nc.scalar.activation(hsq[:, T:2 * T], hc, AF.Square)
                nc.tensor.matmul(p_ss[:], ones_c[:], hsq[:],
                                 start=(ci == 0), stop=(ci == DC - 1))

            def layer_norm(xhat, gb=None, p_ss=None):
                """h_sb (f32) -> xhat (f16 [128, DC*T]) normalized. If gb is
                given, apply per-feature gamma/beta (final LN, f32 out). If
                p_ss is given, the stats were already accumulated inline."""
                if p_ss is None:
                    p_ss = ps_small.tile([1, 2 * T], f32, tag="ps_small")
                    for ci in range(DC):
                        ln_stats_chunk(p_ss, ci)
                m = stat_pool.tile([1, T], f32, tag="st_m")
                msq = stat_pool.tile([1, T], f32, tag="st_msq")
                var = stat_pool.tile([1, T], f32, tag="st_var")
                rstd = stat_pool.tile([1, T], f32, tag="st_rstd")
                mr = stat_pool.tile([1, T], f32, tag="st_mr")
                nc.vector.tensor_scalar_mul(m[:], p_ss[:, 0:T], 1.0 / D)
                nc.vector.tensor_scalar_mul(msq[:], p_ss[:, T:2 * T], 1.0 / D)
                nc.vector.tensor_tensor(out=var[:], in0=m[:], in1=m[:], op=ALU.mult)
                nc.vector.tensor_sub(var[:], msq[:], var[:])
                # rstd = exp(-0.5 * ln(var + eps)) — keeps ACT on the ln/exp table
                nc.scalar.activation(var[:], var[:], AF.Ln, bias=eps_c[:])
                nc.vector.tensor_scalar_mul(var[:], var[:], -0.5)
                nc.scalar.activation(rstd[:], var[:], AF.Exp)
                nc.vector.tensor_tensor(out=mr[:], in0=m[:], in1=rstd[:], op=ALU.mult)
                rstd_b = bc_pool.tile([128, T], f32, tag="rstd_b")
                mr_b = bc_pool.tile([128, T], f32, tag="mr_b")
                nc.gpsimd.partition_broadcast(rstd_b[:], rstd[:])
                nc.gpsimd.partition_broadcast(mr_b[:], mr[:])
                for ci in range(DC):
                    hc = h_sb[:, ci * T:(ci + 1) * T]
                    u = t32_pool.tile([128, T], f32, tag="ln_u")
                    nc.vector.tensor_tensor(out=u[:], in0=hc, in1=rstd_b[:],
                                            op=ALU.mult)
                    if gb is None:
                        nc.vector.tensor_tensor(out=xhat[:, ci * T:(ci + 1) * T],
                                                in0=u[:], in1=mr_b[:],
                                                op=ALU.subtract)
                    else:
                        g_sb, b_sb = gb
                        z = t32_pool.tile([128, T], f32, tag="ln_z")
                        nc.vector.tensor_tensor(out=z[:], in0=u[:], in1=mr_b[:],
                                                op=ALU.subtract)
                        nc.vector.tensor_scalar(
                            out=xhat[:, ci * T:(ci + 1) * T], in0=z[:],
                            scalar1=g_sb[:, ci:ci + 1], scalar2=b_sb[:, ci:ci + 1],
                            op0=ALU.mult, op1=ALU.add,
                        )

            def rope_pair(p_qk, dest, pi, bqk_t, bias_off):
                """p_qk [128,2T] psum holding chunk pair (2pi, 2pi+1); write
                RoPE'd pair into dest[:, 2pi*T:(2pi+2)*T]."""
                qtmp = rope_pool.tile([128, 2 * T], f16, tag="rope_q")
                ctmp = rope_pool.tile([128, 2 * T], f16, tag="rope_c")
                stmp = rope_pool.tile([128, 2 * T], f16, tag="rope_s")
                dtmp = rope_pool.tile([128, 2 * T], f16, tag="rope_d")
                for half in range(2):
                    bcol = bqk_t[:, bias_off + 2 * pi + half:bias_off + 2 * pi + half + 1]
                    sl = slice(half * T, (half + 1) * T)
                    nc.vector.tensor_scalar_add(qtmp[:, sl], p_qk[:, sl], bcol)
                    nc.vector.scalar_tensor_tensor(
                        out=ctmp[:, sl], in0=p_qk[:, sl], scalar=bcol,
                        in1=cos_sb[:, sl], op0=ALU.add, op1=ALU.mult,
                    )
                nc.vector.stream_shuffle(stmp[:], qtmp[:], _SHUF_MASK)
                nc.vector.tensor_tensor(out=dtmp[:], in0=stmp[:], in1=ss_sb[:],
                                        op=ALU.mult)
                nc.vector.tensor_tensor(
                    out=dest[:, 2 * pi * T:(2 * pi + 2) * T],
                    in0=ctmp[:], in1=dtmp[:], op=ALU.add,
                )

            def load_biases(l):
                bias_t = bias_pool.tile([128, 64], f32, tag="biases")
                bv_t = bias_pool.tile([1, D], f16, tag="bv")
                nc.sync.dma_start(out=bias_t[:], in_=biases[l])
                nc.sync.dma_start(out=bv_t[:], in_=bv[l])
                return bias_t, bv_t

            wqk_t, wv_t = load_qkv_weights(0)
            bias_tiles = {0: load_biases(0)}

            for l in range(L):
                # ---- per-layer bias tiles (prefetched) + proj weights ----
                bias_t, bv_t = bias_tiles[l]
                bqk_t = bias_t[:, 0:16]
                bproj_t = bias_t[:, 16:24]
                bfc_t = bias_t[:, 24:56]
                bout_t = bias_t[:, 56:64]
                wproj_t = wpr_pool.tile([128, DC * D], f16, tag="wproj")
                nc.sync.dma_start(
                    out=wproj_t[:].rearrange("p (c n) -> p c n", c=DC),
                    in_=wproj[l].rearrange("(c p) n -> p c n", p=128),
                )

                # ---- LN1 ----
                with nc.named_scope("ln1"):
                    xhat = xh_pool.tile([128, DC * T], f16, tag="xhat")
                    layer_norm(xhat, p_ss=(None if l == 0 else p_ss1))

                # ---- K projection + RoPE (first, so the gather launches early)
                with nc.named_scope("kv_proj"):
                    for pi in range(4):      # k chunk pairs
                        p_qk = ps_bank.tile([128, 2 * T], f32, tag="ps_bank")
                        for half in range(2):
                            fci = HP + 2 * pi + half
                            for dci in range(DC):
                                nc.tensor.matmul(
                                    p_qk[:, half * T:(half + 1) * T],
                                    wqk_t[:, dci * 2 * D + fci * 128:
                                      dci * 2 * D + (fci + 1) * 128],
                                    xhat[:, dci * T:(dci + 1) * T],
                                    start=(dci == 0), stop=(dci == DC - 1),
                                )
                        rope_pair(p_qk, Kl_sb, pi, bqk_t, HP)
                    # v: token-major [T, D] via lhsT = xhat slices
                    for tci in range(2):
                        for fh in range(2):
                            p_v = ps_bank.tile([128, 512], f32, tag="ps_bank")
                            for dci in range(DC):
                                nc.tensor.matmul(
                                    p_v[:],
                                    xhat[:, dci * T + tci * 128: dci * T + (tci + 1) * 128],
                                    wv_t[:, dci * D + fh * 512:
                                         dci * D + (fh + 1) * 512],
                                    start=(dci == 0), stop=False,
                                )
                            nc.tensor.matmul(
                                p_v[:], ones_r[:], bv_t[:, fh * 512:(fh + 1) * 512],
                                start=False, stop=True,
                            )
                            nc.vector.tensor_copy(
                                Vl_sb[:, tci * D + fh * 512: tci * D + (fh + 1) * 512],
                                p_v[:],
                            )

                # ---- stage K^T,V to DRAM; AllGather within batch group ----
                with nc.named_scope("kv_gather"):
                    kvloc_k = kvloc[0:KV_K].rearrange("(p f) -> p f", p=128)
                    for pi in range(4):
                        nc.sync.dma_start(
                            out=kvloc_k[:, 2 * pi * T:(2 * pi + 2) * T],
                            in_=Kl_sb[:, 2 * pi * T:(2 * pi + 2) * T],
                        )
                    nc.sync.dma_start(
                        out=kvloc[KV_K:KV_TOT].rearrange(
                            "(c p f) -> p c f", p=128, f=D),
                        in_=Vl_sb[:].rearrange("p (c f) -> p c f", f=D),
                    )
                    nc.gpsimd.collective_compute(
                        "AllGather",
                        ALU.bypass,
                        ins=[kvloc.opt()],
                        outs=[kvag.opt()],
                        replica_groups=GROUPS,
                    )

                def issue_wfc(g):
                    t = wfc_pool.tile([128, DC * 512], f16, tag="wfc")
                    nc.sync.dma_start(
                        out=t[:].rearrange("p (c n) -> p c n", c=DC),
                        in_=wfc[l].rearrange("(c p) n -> p c n", p=128)
                            [:, :, g * 512:(g + 1) * 512],
                    )
                    return t

                def issue_wout(i):
                    half, fcg = i // 4, i % 4
                    t = wout_pool.tile([128, 8 * 512], f16, tag="wout")
                    nc.sync.dma_start(
                        out=t[:].rearrange("p (c n) -> p c n", c=8),
                        in_=wout[l].rearrange("(c p) n -> p c n", p=128)
                            [:, fcg * 8:(fcg + 1) * 8,
                             half * 512:(half + 1) * 512],
                    )
                    return t

                wfc_ts = {g: issue_wfc(g) for g in range(3)}
                wout_ts = {i: issue_wout(i) for i in range(2)}

                # ---- Q projection + RoPE (overlaps the gather) ----
                with nc.named_scope("q_proj"):
                    for pi in range(4):      # q chunk pairs
                        p_qk = ps_bank.tile([128, 2 * T], f32, tag="ps_bank")
                        for half in range(2):
                            fci = 2 * pi + half
                            for dci in range(DC):
                                nc.tensor.matmul(
                                    p_qk[:, half * T:(half + 1) * T],
                                    wqk_t[:, dci * 2 * D + fci * 128:
                                      dci * 2 * D + (fci + 1) * 128],
                                    xhat[:, dci * T:(dci + 1) * T],
                                    start=(dci == 0), stop=(dci == DC - 1),
                                )
                        rope_pair(p_qk, Q_sb, pi, bqk_t, 0)

                # ---- unpack gathered K/V ----
                with nc.named_scope("kv_unpack"):
                    nc.sync.dma_start(
                        out=K_sb[:].rearrange("p (r f) -> p r f", r=RANKS),
                        in_=kvag.rearrange("(r x) -> r x", r=RANKS)
                            [:, 0:KV_K].rearrange("r (p f) -> p r f", p=128),
                    )
                    for rr in range(RANKS):
                        base = rr * KV_TOT
                        for tci in range(2):
                            vbase = base + KV_K + tci * D * 128
                            nc.sync.dma_start(
                                out=V_sb[:].rearrange(
                                    "p (c h x) -> p c h x", h=H, x=VW
                                )[:, 2 * rr + tci, :, 0:DH],
                                in_=kvag[vbase:vbase + D * 128].rearrange(
                                    "(p h d) -> p h d", p=128, d=DH),
                            )

                # ---- attention ----
                with nc.named_scope("attn"):
                    for hp in range(HP):
                        p_at = ps_at.tile([VW, 2 * T], f32, tag="ps_at")
                        for hh in range(2):
                            bp = hh * 64
                            hglob = 2 * hp + hh
                            for kp in range(DC // 2):
                                p_s = ps_bank.tile([128, 2 * T], f32, tag="ps_bank")
                                for half in range(2):
                                    kc = 2 * kp + half
                                    koff = ((kc // 2) * HP * T + hp * T
                                            + (kc % 2) * 128)
                                    nc.tensor.matmul(
                                        p_s[:, half * T:(half + 1) * T],
                                        K_sb[bp:bp + 64, koff:koff + 128],
                                        Q_sb[bp:bp + 64, hp * T:(hp + 1) * T],
                                        start=True, stop=True,
                                    )
                                probs = probs_pool.tile([128, 2 * T], f16, tag="probs")
                                nc.scalar.activation(probs[:], p_s[:], AF.Exp)
                                nc.vector.tensor_tensor(
                                    out=probs[:], in0=probs[:],
                                    in1=mask_sb[:, 2 * kp * T:(2 * kp + 2) * T],
                                    op=ALU.mult,
                                )
                                for half in range(2):
                                    kc = 2 * kp + half
                                    nc.tensor.matmul(
                                        p_at[0:VW, hh * T:(hh + 1) * T],
                                        V_sb[:, kc * H * VW + hglob * VW:
                                             kc * H * VW + (hglob + 1) * VW],
                                        probs[:, half * T:(half + 1) * T],
                                        start=(kc == 0), stop=(kc == DC - 1),
                                    )
                        for hh in range(2):
                            recip = stat_pool.tile([1, T], f32, tag="recip")
                            nc.vector.reciprocal(
                                recip[:], p_at[DH:DH + 1, hh * T:(hh + 1) * T])
                            rb = bc_pool.tile([64, T], f32, tag="rb")
                            nc.gpsimd.partition_broadcast(rb[:], recip[:])
                            nc.vector.tensor_tensor(
                                out=attn_sb[hh * 64:(hh + 1) * 64,
                                            hp * T:(hp + 1) * T],
                                in0=p_at[0:64, hh * T:(hh + 1) * T],
                                in1=rb[:],
                                op=ALU.mult,
                            )

                # ---- attention out-proj + residual ----
                with nc.named_scope("proj"):
                    p_ss2 = ps_small.tile([1, 2 * T], f32, tag="ps_small")
                    for half in range(2):
                        p_pr = [ps_bank.tile([128, 2 * T], f32, tag="ps_bank",
                                             name=f"p_pr{dj}") for dj in range(2)]
                        for dj in range(4):
                            for cin in range(DC):
                                off = cin * D + half * 512 + dj * 128
                                nc.tensor.matmul(
                                    p_pr[dj // 2][:, (dj % 2) * T:(dj % 2 + 1) * T],
                                    wproj_t[:, off:off + 128],
                                    attn_sb[:, cin * T:(cin + 1) * T],
                                    start=(cin == 0), stop=(cin == DC - 1),
                                )
                        for dj in range(4):
                            dci = half * 4 + dj
                            nc.vector.scalar_tensor_tensor(
                                out=h_sb[:, dci * T:(dci + 1) * T],
                                in0=p_pr[dj // 2][:, (dj % 2) * T:(dj % 2 + 1) * T],
                                scalar=bproj_t[:, dci:dci + 1],
                                in1=h_sb[:, dci * T:(dci + 1) * T],
                                op0=ALU.add, op1=ALU.add,
                            )
                            ln_stats_chunk(p_ss2, dci)

                # ---- LN2 ----
                with nc.named_scope("ln2"):
                    xhat2 = xh_pool.tile([128, DC * T], f16, tag="xhat")
                    layer_norm(xhat2, p_ss=p_ss2)

                # prefetch next layer's qkv weights + biases (overlaps the FFN)
                if l + 1 < L:
                    wqk_t, wv_t = load_qkv_weights(l + 1)
                    bias_tiles[l + 1] = load_biases(l + 1)

                # ---- FFN: fc + gelu -> h1, then out-proj + residual ----
                with nc.named_scope("fc"):
                    for g in range(F // 512):          # 8 groups of 4 output chunks
                        p_fc = [ps_bank.tile([128, 2 * T], f32, tag="ps_bank",
                                             name=f"p_fc{fj}") for fj in range(2)]
                        wfc_t = wfc_ts[g]
                        if g + 3 < F // 512:
                            wfc_ts[g + 3] = issue_wfc(g + 3)
                        for fj in range(4):
                            for dci in range(DC):
                                nc.tensor.matmul(
                                    p_fc[fj // 2][:, (fj % 2) * T:(fj % 2 + 1) * T],
                                    wfc_t[:, dci * 512 + fj * 128:
                                          dci * 512 + (fj + 1) * 128],
                                    xhat2[:, dci * T:(dci + 1) * T],
                                    start=(dci == 0), stop=(dci == DC - 1),
                                )
                        for fj in range(4):
                            fci = g * 4 + fj
                            nc.scalar.activation(
                                h1_sb[:, fci * T:(fci + 1) * T],
                                p_fc[fj // 2][:, (fj % 2) * T:(fj % 2 + 1) * T],
                                AF.Gelu_apprx_tanh,
                                bias=bfc_t[:, fci:fci + 1],
                            )
                with nc.named_scope("ffn_out"):
                    warm = stat_pool.tile([1, 1], f32, tag="warm")
                    nc.scalar.activation(warm[:], eps_c[:], AF.Ln)
                    p_ss1 = ps_small.tile([1, 2 * T], f32, tag="ps_small")
                    for half in range(2):
                        p_o = [ps_bank.tile([128, 2 * T], f32, tag="ps_bank",
                                            name=f"p_o{dj}") for dj in range(4)]
                        for fcg in range(4):       # 32 contraction chunks in 4 groups
                            i = half * 4 + fcg
                            wout_t = wout_ts[i]
                            if i + 2 < 8:
                                wout_ts[i + 2] = issue_wout(i + 2)
                            for fcl in range(8):
                                fci = fcg * 8 + fcl
                                for dj in range(4):
                                    nc.tensor.matmul(
                                        p_o[dj][:, 0:T],
                                        wout_t[:, fcl * 512 + dj * 128:
                                               fcl * 512 + (dj + 1) * 128],
                                        h1_sb[:, fci * T:(fci + 1) * T],
                                        start=(fci == 0), stop=(fci == F // 128 - 1),
                                    )
                        for dj in range(4):
                            dci = half * 4 + dj
                            nc.vector.scalar_tensor_tensor(
                                out=h_sb[:, dci * T:(dci + 1) * T],
                                in0=p_o[dj][:, 0:T],
                                scalar=bout_t[:, dci:dci + 1],
                                in1=h_sb[:, dci * T:(dci + 1) * T],
                                op0=ALU.add, op1=ALU.add,
                            )
                            ln_stats_chunk(p_ss1, dci)

            # ---- final LN with gamma/beta, fp32 apply (in place in h_sb) ----
            with nc.named_scope("final_ln"):
                layer_norm(h_sb, gb=(lnfg_sb, lnfb_sb), p_ss=p_ss1)
                nc.sync.dma_start(
                    out=outT.rearrange("(c p) t -> p c t", p=128),
                    in_=h_sb[:].rearrange("p (c t) -> p c t", t=T),
                )

    nc.compile()
    return nc


_CACHED = {}


def _prep_inputs(inputs_embeds, w_qkv, b_qkv, w_proj, b_proj, w_fc, b_fc,
                 w_out, b_out, ln1_g, ln1_b, ln2_g, ln2_b, lnf_g, lnf_b):
    """Fold LN gamma/beta into weights; permute+scale q/k; cast to fp16."""
    perm = _qk_perm()
    f16 = np.float16
    qs = 0.125 / KV_SCALE
    wqk_l, wv_l, bqk_l, bv_l = [], [], [], []
    wfc_l, bfc_l = [], []
    for l in range(L):
        b_eff = b_qkv[l] + ln1_b[l] @ w_qkv[l]          # [3D]
        w_eff = ln1_g[l][:, None] * w_qkv[l]            # [D, 3D]
        wq = w_eff[:, perm] * qs
        wk = w_eff[:, D + perm] * KV_SCALE
        bq = b_eff[perm] * qs
        bk = b_eff[D + perm] * KV_SCALE
        wqk_l.append(np.concatenate([wq, wk], axis=1).astype(f16))
        wv_l.append((w_eff[:, 2 * D:] * KV_SCALE).astype(f16))
        bqk_l.append(np.concatenate([bq, bk]).reshape(16, 128).T.astype(np.float32))
        # packed later
        bv_l.append((b_eff[2 * D:] * KV_SCALE).reshape(1, D).astype(f16))
        bfc_eff = b_fc[l] + ln2_b[l] @ w_fc[l]
        wfc_l.append((ln2_g[l][:, None] * w_fc[l]).astype(f16))
        bfc_l.append(bfc_eff.reshape(32, 128).T.astype(np.float32))
    bproj_p = b_proj.reshape(L, 8, 128).transpose(0, 2, 1).astype(np.float32)
    bout_p = b_out.reshape(L, 8, 128).transpose(0, 2, 1).astype(np.float32)
    biases_p = np.concatenate(
        [np.stack(bqk_l), bproj_p, np.stack(bfc_l), bout_p], axis=2
    ).astype(np.float32)                                   # [L, 128, 64]
    shared = {
        "wqk": np.stack(wqk_l),
        "wv": np.stack(wv_l),
        "wproj": w_proj.astype(f16),
        "wfc": np.stack(wfc_l),
        "wout": w_out.astype(f16),
        "biases": biases_p,
        "bv": np.stack(bv_l),
        "lnfg": lnf_g.reshape(8, 128).T.astype(np.float32),
        "lnfb": lnf_b.reshape(8, 128).T.astype(np.float32),
    }
    x_flat = np.asarray(inputs_embeds, dtype=np.float32).reshape(B * S, D)
    in_maps = []
    for c in range(N_CORES):
        cos2, ss2 = _rope_tables(c)
        m = dict(shared)
        m["x0T"] = np.ascontiguousarray(x_flat[c * T:(c + 1) * T].T)
        m["cosT"] = cos2
        m["ssT"] = ss2
        m["maskT"] = _causal_mask(c)
        in_maps.append(m)
    return in_maps


def kernel(**inputs):
    inputs = {k: np.asarray(v) for k, v in inputs.items()}
    in_maps = _prep_inputs(
        inputs["inputs_embeds"], inputs["w_qkv"], inputs["b_qkv"],
        inputs["w_proj"], inputs["b_proj"], inputs["w_fc"], inputs["b_fc"],
        inputs["w_out"], inputs["b_out"], inputs["ln1_g"], inputs["ln1_b"],
        inputs["ln2_g"], inputs["ln2_b"], inputs["lnf_g"], inputs["lnf_b"],
    )
    if "nc" not in _CACHED:
        _CACHED["nc"] = build_program()
    res = run_bass_kernel_spmd(_CACHED["nc"], in_maps, list(range(N_CORES)))
    out = np.empty((B * S, D), dtype=np.float32)
    for c in range(N_CORES):
        out[c * T:(c + 1) * T] = res.results[c]["outT"].T
    return out.reshape(B, S, D)


if __name__ == "__main__":
    print("building program...")
    build_program()
    print("built OK")
